# revision 6
# baseline (speedup 1.0000x reference)
"""Multi-head attention (B=2, S=2048, D=1024, H=16) on 8 NeuronCores.

Sharding: core c -> (batch b = c//4, head-group hg = c%4 of 4 heads).
Each core computes QKV projection for its 4 heads (bf16 matmuls, f32 PSUM),
transposed-score flash attention (S^T = K^T-tile.T-stationary @ Q^T streams,
softmax denominator via an appended ones-column on V), and the output
projection restricted to its heads' rows of out_w.  The host sums the 4
per-head-group partial outputs per batch and adds out_b (exact, linear).

Schedule: one continuous PE-dense pipeline.  The PE clock gate (HAM) holds
K=8/8 only while the PE is near-100% busy per 3.4us window, so the ACT-bound
softmax era is woven with the remaining projection work, the first half's
output projection, and a small junk-matmul trickle to keep the PE dense.

Device layouts (per core):
  xt  [D(+1), S]  bf16   x[b]^T (+ ones row when qkv_b != 0)
  w   [D(+1), 768] bf16  qkv_w columns for this core's heads (q|k|v) (+ bias row)
  wo  [256, D] bf16      out_w rows for this core's heads
  out [S, D] bf16        partial output (sum over the 4 head-groups = x-slice
                         contribution; host adds groups + out_b)
"""

import os
import sys
from collections import deque

sys.path.insert(0, "/opt/trn_rl_repo")

import numpy as np
import ml_dtypes

import concourse.bass as bass  # noqa: F401  (AP helpers)
import concourse.mybir as mybir
import concourse.tile as tile
from concourse import bacc
from concourse.bass_utils import run_bass_kernel_spmd
from concourse.masks import make_upper_triangular

B, S, D, H, DH = 2, 2048, 1024, 16, 64
NCORES = 8
HPC = 4            # heads per core
EQ = HPC * DH      # 256: q (or k, or v) columns per core
E = 3 * EQ         # 768: total projected columns per core
BF16 = mybir.dt.bfloat16
F32 = mybir.dt.float32
NP_BF16 = ml_dtypes.bfloat16
EXPFN = mybir.ActivationFunctionType.Exp
HQ = S // 2        # 1024 queries per half

JUNK_PER_GROUP = 2   # PE-density trickle inside the softmax pipeline

_prog_cache: dict = {}
last_results = None  # BassKernelResults of the most recent run (for test.py)


def _build_groups(Q0, Q1, causal):
    """Score chunks for one (head, half): (j, qoff, clen) with chunk
    boundaries on ctx 512-col banks, paired into equal-length exp groups."""
    chunks = []
    for j in range(16):
        if causal and 128 * j >= Q1:
            break
        q0 = max(128 * j, Q0) if causal else Q0
        off = q0
        while off < Q1:
            nxt = Q0 + ((off - Q0) // 512 + 1) * 512
            clen = min(nxt, Q1) - off
            chunks.append((j, off, clen))
            off += clen
    buckets = {}
    for c in chunks:
        buckets.setdefault(c[2], []).append(c)
    groups = []
    for L in sorted(buckets, reverse=True):
        lst = buckets[L]
        for k in range(0, len(lst) - 1, 2):
            groups.append((lst[k], lst[k + 1]))
        if len(lst) % 2:
            groups.append((lst[-1],))
    return groups


def _emit(tc, xt_h, w_h, wo_h, out_h, causal, dd):
    nc = tc.nc
    nd = (dd + 127) // 128          # number of contraction sub-tiles
    dsubs = [(i * 128, min(128, dd - i * 128)) for i in range(nd)]

    with (
        tc.tile_pool(name="persist", bufs=1) as pp,
        tc.tile_pool(name="pt", bufs=1) as pt_pool,
        tc.tile_pool(name="norm", bufs=2) as norm_pool,
        tc.tile_pool(name="outsb", bufs=3) as out_pool,
        tc.tile_pool(name="dram", bufs=2, space="DRAM") as dram_pool,
    ):
        # ---- persistent SBUF tensors ----
        xt_sb = pp.tile([128, nd, S], BF16, tag="xt", name="xt_sb")
        w_sb = pp.tile([128, nd, E], BF16, tag="w", name="w_sb")
        wo_sb = pp.tile([128, 2, D], BF16, tag="wo", name="wo_sb")
        qkT_sb = pp.tile([128, 4, S], BF16, tag="qkT", name="qkT_sb")
        # V' per (k-tile j, head h): [128, 65], col 64 = ones (softmax denom)
        vp_sb = pp.tile([128, 16, HPC, 65], BF16, tag="vp", name="vp_sb")
        ctx_all = pp.tile([128, 2, S], BF16, tag="ctx", name="ctx_all")

        warm_sb = pp.tile([128, 128], BF16, tag="warm", name="warm_sb")
        nc.vector.memset(warm_sb, 0.0)
        # Pre-load the exp activation table set now (~2.7us); doing it
        # lazily at the first softmax exp opens a >3.4us PE-idle window at
        # the proj->attention boundary and re-throttles the PE clock.
        exp_pre = pp.tile([128, 8], F32, tag="exppre", name="exp_pre")
        nc.scalar.activation(exp_pre, warm_sb[:, 0:8], EXPFN)

        # d-major DMA bundles to match the e-tile d-loop consumption order;
        # full-row xt transfers give 4KB per-partition packets.
        for d, (o, ln) in enumerate(dsubs):
            nc.sync.dma_start(out=w_sb[0:ln, d, :], in_=w_h[o : o + ln, :])
            nc.sync.dma_start(out=xt_sb[0:ln, d, :], in_=xt_h[o : o + ln, :])
        for et in range(2):
            nc.sync.dma_start(
                out=wo_sb[:, et, :], in_=wo_h[128 * et : 128 * (et + 1), :]
            )

        if causal:
            tri_f = pp.tile([128, 128], F32, tag="trif", name="tri_f")
            make_upper_triangular(nc, tri_f, val=1.0, diag=True)
            tri_bf = pp.tile([128, 128], BF16, tag="trib", name="tri_bf")
            nc.vector.tensor_copy(tri_bf, tri_f)

        nc.vector.memset(vp_sb[:, :, :, 64:65], 1.0)

        # ---- era 1: PE warmup on constant data while input DMA streams ----
        with tc.tile_pool(name="wup", bufs=1, space="PSUM") as wup:
            wps = wup.tile([128, 128], F32, tag="w", name="warm_ps")
            for _ in range(96):
                nc.tensor.matmul(wps, lhsT=warm_sb, rhs=warm_sb,
                                 start=True, stop=True)

        # ---- era 2: Q/K proj for pair 0 (e-tiles 0,2) d-outer (DMA-paced),
        #      then V s-tiles 0..7 (enough to unlock half-0 attention) ----
        def emit_v_tile(i, ps, evac_eng):
            for d in range(nd):
                ln = dsubs[d][1]
                nc.tensor.matmul(
                    ps,
                    lhsT=xt_sb[0:ln, d, 128 * i : 128 * (i + 1)],
                    rhs=w_sb[0:ln, d, 2 * EQ : 3 * EQ],
                    start=(d == 0),
                    stop=(d == nd - 1),
                )
            evac_eng(
                vp_sb[:, i, :, 0:64],
                ps.rearrange("p (h e) -> p h e", h=HPC),
            )

        with tc.tile_pool(name="pj", bufs=1, space="PSUM") as filp:
            pss = {}
            for et in (0, 2):
                for ch in range(4):
                    pss[(et, ch)] = filp.tile(
                        [128, 512], F32, tag=f"p{et}c{ch}", name=f"p{et}c{ch}"
                    )
            for d in range(nd):
                ln = dsubs[d][1]
                for et in (0, 2):
                    for ch in range(4):
                        nc.tensor.matmul(
                            pss[(et, ch)],
                            lhsT=w_sb[0:ln, d, 128 * et : 128 * (et + 1)],
                            rhs=xt_sb[0:ln, d, 512 * ch : 512 * (ch + 1)],
                            start=(d == 0),
                            stop=(d == nd - 1),
                        )
            for ch in range(4):
                nc.scalar.copy(
                    qkT_sb[:, 0, 512 * ch : 512 * (ch + 1)], pss[(0, ch)]
                )
                nc.vector.tensor_copy(
                    qkT_sb[:, 2, 512 * ch : 512 * (ch + 1)], pss[(2, ch)]
                )
            for i in range(8):
                vps = filp.tile([128, 256], F32, tag=f"p0c{i % 4}", name="v_ps")
                emit_v_tile(
                    i, vps, nc.scalar.copy if i % 2 else nc.vector.tensor_copy
                )

        # ---- era 3: softmax pipeline woven with leftover proj + outproj(h0) ----
        with (
            tc.tile_pool(name="stp", bufs=1, space="PSUM") as stp,
            tc.tile_pool(name="ctxp", bufs=1, space="PSUM") as ctxp,
            tc.tile_pool(name="wvp", bufs=1, space="PSUM") as wvp,
            tc.tile_pool(name="jkp", bufs=1, space="PSUM") as jkp,
        ):
            junk_t = jkp.tile([128, 128], F32, tag="jk", name="junk_ps")

            def weave_qk(et):
                """One 512-col chunk of Q/K proj for e-tile et (pair 1)."""
                for ch in range(4):
                    ps = wvp.tile([128, 512], F32, tag="wv", name="wv_ps")
                    for d in range(nd):
                        ln = dsubs[d][1]
                        nc.tensor.matmul(
                            ps,
                            lhsT=w_sb[0:ln, d, 128 * et : 128 * (et + 1)],
                            rhs=xt_sb[0:ln, d, 512 * ch : 512 * (ch + 1)],
                            start=(d == 0),
                            stop=(d == nd - 1),
                        )
                    nc.vector.tensor_copy(
                        qkT_sb[:, et, 512 * ch : 512 * (ch + 1)], ps
                    )
                    yield

            def weave_v(lo, hi):
                for i in range(lo, hi):
                    ps = wvp.tile([128, 256], F32, tag="wv", name="wv_ps")
                    emit_v_tile(i, ps, nc.vector.tensor_copy)
                    yield

            def weave_outproj(lo, hi):
                """Output projection for query tiles [lo, hi) through the
                1-bank weave slot; needs ctx_all of both pairs normalized."""
                for i in range(lo, hi):
                    osb = out_pool.tile([128, D], BF16, tag="osb", name="o_sb")
                    for c in range(2):
                        ps = wvp.tile([128, 512], F32, tag="wv", name="wv_ps")
                        for et in range(2):
                            nc.tensor.matmul(
                                ps,
                                lhsT=ctx_all[:, et, 128 * i : 128 * (i + 1)],
                                rhs=wo_sb[:, et, 512 * c : 512 * (c + 1)],
                                start=(et == 0),
                                stop=(et == 1),
                            )
                        nc.vector.tensor_copy(osb[:, 512 * c : 512 * (c + 1)], ps)
                        yield
                    nc.sync.dma_start(
                        out=out_h[128 * i : 128 * (i + 1), :], in_=osb
                    )

            def attn_head(p, hh, half, weave_iter):
                hidx = 2 * p + hh
                ksl = qkT_sb[hh * 64 : (hh + 1) * 64, 2 + p, :]
                qsl = qkT_sb[hh * 64 : (hh + 1) * 64, p, :]
                Q0, Q1 = HQ * half, HQ * (half + 1)
                groups = _build_groups(Q0, Q1, causal)
                ctx_ps = ctxp.tile([65, HQ], F32, tag="ctx", name="ctx_ps")
                st = stp.tile([128, 4, 512], F32, tag="st", name="st_ps")
                pts = pt_pool.tile([128, 4, 512], BF16, tag="pts", name="pts")

                bank_cnt = [0, 0]
                for grp in groups:
                    for (j, qoff, clen) in grp:
                        bank_cnt[(qoff - Q0) // 512] += 1
                bank_started = [False, False]

                def emit_avs(grp, slots):
                    for (j, qoff, clen), slot in zip(grp, slots):
                        b = (qoff - Q0) // 512
                        first = not bank_started[b]
                        bank_started[b] = True
                        bank_cnt[b] -= 1
                        nc.tensor.matmul(
                            ctx_ps[:, qoff - Q0 : qoff - Q0 + clen],
                            lhsT=vp_sb[:, j, hidx, :],
                            rhs=pts[:, slot, 0:clen],
                            start=first,
                            stop=(bank_cnt[b] == 0),
                        )

                pend = deque()
                for gi, grp in enumerate(groups):
                    base = 2 * (gi % 2)
                    slots = (base, base + 1)[: len(grp)]
                    L = grp[0][2]
                    for (j, qoff, clen), slot in zip(grp, slots):
                        nc.tensor.matmul(
                            st[:, slot, 0:clen],
                            lhsT=ksl[:, 128 * j : 128 * (j + 1)],
                            rhs=qsl[:, qoff : qoff + clen],
                            start=True,
                            stop=True,
                        )
                    n = len(grp)
                    nc.scalar.activation(
                        pts[:, base : base + n, 0:L],
                        st[:, base : base + n, 0:L],
                        EXPFN,
                        scale=0.125,
                    )
                    if causal:
                        for (j, qoff, clen), slot in zip(grp, slots):
                            if 128 * j >= Q0 and qoff == 128 * j:
                                nc.gpsimd.tensor_mul(
                                    pts[:, slot, 0:128],
                                    pts[:, slot, 0:128],
                                    tri_bf,
                                )
                    pend.append((grp, slots))
                    if len(pend) >= 2:
                        emit_avs(*pend.popleft())
                    if weave_iter is not None:
                        next(weave_iter, None)
                    for _ in range(JUNK_PER_GROUP):
                        nc.tensor.matmul(
                            junk_t, lhsT=warm_sb, rhs=warm_sb,
                            start=True, stop=True,
                        )
                while pend:
                    emit_avs(*pend.popleft())

                # normalization: evacuate unnormalized ctx^T (+ denom row 64),
                # reshape the denom row via a DRAM bounce to use all DVE
                # lanes for the reciprocal, broadcast back, multiply.
                ctxu = norm_pool.tile([65, HQ], F32, tag="ctxu", name="ctxu")
                nc.vector.tensor_copy(ctxu, ctx_ps)
                den_d = dram_pool.tile([HQ], F32, tag="dend", name="den_d")
                nc.sync.dma_start(out=den_d, in_=ctxu[64:65, :])
                den_sp = norm_pool.tile(
                    [128, HQ // 128], F32, tag="densp", name="den_sp"
                )
                nc.sync.dma_start(
                    out=den_sp, in_=den_d.rearrange("(p i) -> p i", p=128)
                )
                rec_sp = norm_pool.tile(
                    [128, HQ // 128], F32, tag="recsp", name="rec_sp"
                )
                nc.vector.reciprocal(rec_sp, den_sp)
                rec_d = dram_pool.tile([HQ], F32, tag="recd", name="rec_d")
                nc.sync.dma_start(out=rec_d, in_=rec_sp)
                recb = norm_pool.tile([64, HQ], F32, tag="recb", name="recb")
                rec_bcast = bass.AP(
                    tensor=rec_d.tensor, offset=rec_d.offset,
                    ap=[[0, 64]] + list(rec_d.ap),
                )
                nc.sync.dma_start(out=recb, in_=rec_bcast)
                nc.vector.tensor_mul(
                    ctx_all[64 * hh : 64 * hh + 64, p, Q0:Q1],
                    ctxu[0:64, :],
                    recb,
                )

            qk13 = (w for et in (1, 3) for w in weave_qk(et))
            v_hi = weave_v(8, 16)
            op_h0 = weave_outproj(0, 8)
            # half 0: weave pair-1 Q/K proj, then V 8..15
            attn_head(0, 0, 0, qk13)
            attn_head(0, 1, 0, qk13)
            attn_head(1, 0, 0, v_hi)
            attn_head(1, 1, 0, v_hi)
            # half 1: weave the half-0 output projection
            attn_head(0, 0, 1, op_h0)
            attn_head(0, 1, 1, op_h0)
            attn_head(1, 0, 1, op_h0)
            attn_head(1, 1, 1, op_h0)
            for _ in op_h0:   # drain any leftover outproj chunks
                pass
            for it in (qk13, v_hi):
                for _ in it:
                    pass

        # ---- era 4: output projection for half 1 ----
        with tc.tile_pool(name="op", bufs=3, space="PSUM") as op:
            for i in range(8, 16):
                ops = op.tile([128, D], F32, tag="o", name="o_ps")
                for c in range(2):
                    for et in range(2):
                        nc.tensor.matmul(
                            ops[:, 512 * c : 512 * (c + 1)],
                            lhsT=ctx_all[:, et, 128 * i : 128 * (i + 1)],
                            rhs=wo_sb[:, et, 512 * c : 512 * (c + 1)],
                            start=(et == 0),
                            stop=(et == 1),
                        )
                osb = out_pool.tile([128, D], BF16, tag="osb", name="o_sb")
                if i % 2:
                    nc.scalar.copy(osb, ops)
                else:
                    nc.vector.tensor_copy(osb, ops)
                nc.sync.dma_start(out=out_h[128 * i : 128 * (i + 1), :], in_=osb)


def _get_prog(causal: bool, dd: int):
    key = (causal, dd)
    if key not in _prog_cache:
        nc = bacc.Bacc("TRN2", target_bir_lowering=False, debug=False)
        xt_h = nc.dram_tensor("xt", [dd, S], BF16, kind="ExternalInput")
        w_h = nc.dram_tensor("w", [dd, E], BF16, kind="ExternalInput")
        wo_h = nc.dram_tensor("wo", [EQ, D], BF16, kind="ExternalInput")
        out_h = nc.dram_tensor("out", [S, D], BF16, kind="ExternalOutput")
        with tile.TileContext(nc) as tc:
            _emit(tc, xt_h, w_h, wo_h, out_h, causal, dd)
        nc.compile()
        _prog_cache[key] = nc
    return _prog_cache[key]


def _numpy_fallback(x, mask, qkv_w, qkv_b, out_w, out_b):
    qkv = x.reshape(B * S, D) @ qkv_w + qkv_b
    qkv = qkv.reshape(B, S, 3, H, DH)
    q, k, v = qkv[:, :, 0], qkv[:, :, 1], qkv[:, :, 2]
    sc = np.einsum("bqhd,bkhd->bhqk", q, k) / np.sqrt(np.float32(DH))
    sc = np.where(mask, sc, np.float32(-1e9))
    sc = sc - sc.max(-1, keepdims=True)
    a = np.exp(sc)
    a = a / a.sum(-1, keepdims=True)
    ctx = np.einsum("bhqk,bkhd->bqhd", a, v).reshape(B, S, D)
    return (ctx.reshape(B * S, D) @ out_w + out_b).reshape(B, S, D).astype(np.float32)


def kernel(x, mask, qkv_w, qkv_b, out_w, out_b):
    global last_results
    x = np.asarray(x, dtype=np.float32)
    mask = np.asarray(mask).astype(bool)
    qkv_w = np.asarray(qkv_w, dtype=np.float32)
    qkv_b = np.asarray(qkv_b, dtype=np.float32)
    out_w = np.asarray(out_w, dtype=np.float32)
    out_b = np.asarray(out_b, dtype=np.float32)

    m2 = mask.reshape(S, S)
    if m2.all():
        causal = False
    elif np.array_equal(m2, np.tril(np.ones((S, S), dtype=bool))):
        causal = True
    else:
        return _numpy_fallback(x, mask, qkv_w, qkv_b, out_w, out_b)

    has_b = bool(np.any(qkv_b))
    dd = D + 1 if has_b else D
    nc = _get_prog(causal, dd)

    in_maps = []
    for c in range(NCORES):
        b, hg = divmod(c, 4)
        hs = hg * HPC
        cols = slice(hs * DH, (hs + HPC) * DH)
        wc = np.concatenate(
            [qkv_w[:, cols], qkv_w[:, D:][:, cols], qkv_w[:, 2 * D :][:, cols]], axis=1
        )
        xtc = x[b].T
        if has_b:
            bc = np.concatenate(
                [qkv_b[cols], qkv_b[D:][cols], qkv_b[2 * D :][cols]]
            )
            wc = np.concatenate([wc, bc[None, :]], axis=0)
            xtc = np.concatenate([xtc, np.ones((1, S), np.float32)], axis=0)
        in_maps.append(
            {
                "xt": np.ascontiguousarray(xtc).astype(NP_BF16),
                "w": np.ascontiguousarray(wc).astype(NP_BF16),
                "wo": np.ascontiguousarray(out_w[cols, :]).astype(NP_BF16),
            }
        )

    trace = os.environ.get("KERNEL_TRACE", "0") == "1"
    last_results = run_bass_kernel_spmd(
        nc, in_maps, core_ids=list(range(NCORES)), trace=trace
    )
    out = np.zeros((B, S, D), dtype=np.float32)
    for c in range(NCORES):
        out[c // 4] += np.asarray(last_results.results[c]["out"], dtype=np.float32)
    out += out_b[None, None, :]
    return out


# revision 49
# speedup vs baseline: 1.3075x; 1.3075x over previous
"""Multi-head attention (B=2, S=2048, D=1024, H=16) on 8 NeuronCores.

Sharding: core c -> (batch b = c//4, head-group hg = c%4 of 4 heads).
Each core computes QKV projection for its 4 heads (bf16 matmuls, f32 PSUM),
transposed-score flash attention (S^T = K^T-tile.T-stationary @ Q^T streams,
softmax denominator via an appended ones-column on V), and the output
projection restricted to its heads' rows of out_w.  The host sums the 4
per-head-group partial outputs per batch and adds out_b (exact, linear).

Schedule: one continuous PE-dense pipeline.  The PE clock gate (HAM) holds
K=8/8 only while the PE is near-100% busy per 3.4us window, so the ACT-bound
softmax era is woven with the remaining projection work, the first half's
output projection, and a small junk-matmul trickle to keep the PE dense.

Device layouts (per core):
  xt  [D(+1), S]  bf16   x[b]^T (+ ones row when qkv_b != 0)
  w   [D(+1), 768] bf16  qkv_w columns for this core's heads (q|k|v) (+ bias row)
  wo  [256, D] bf16      out_w rows for this core's heads
  out [S, D] bf16        partial output (sum over the 4 head-groups = x-slice
                         contribution; host adds groups + out_b)
"""

import os
import sys
from collections import deque

sys.path.insert(0, "/opt/trn_rl_repo")

import numpy as np
import ml_dtypes

import concourse.bass as bass  # noqa: F401  (AP helpers)
import concourse.mybir as mybir
import concourse.tile as tile
from concourse import bacc
from concourse.bass_utils import run_bass_kernel_spmd
from concourse.masks import make_upper_triangular

B, S, D, H, DH = 2, 2048, 1024, 16, 64
NCORES = 8
HPC = 4            # heads per core
EQ = HPC * DH      # 256: q (or k, or v) columns per core
E = 3 * EQ         # 768: total projected columns per core
BF16 = mybir.dt.bfloat16
F32 = mybir.dt.float32
NP_BF16 = ml_dtypes.bfloat16
EXPFN = mybir.ActivationFunctionType.Exp
HQ = S // 2        # 1024 queries per half

JUNK_PER_GROUP = 2   # PE-density trickle inside the softmax pipeline

_prog_cache: dict = {}
last_results = None  # BassKernelResults of the most recent run (for test.py)


def _build_chunks(Q0, Q1, causal):
    """Score chunks for one (head, half): (j, qoff, clen) with chunk
    boundaries on ctx 512-col banks, j ascending."""
    chunks = []
    for j in range(16):
        if causal and 128 * j >= Q1:
            break
        q0 = max(128 * j, Q0) if causal else Q0
        off = q0
        while off < Q1:
            nxt = Q0 + ((off - Q0) // 512 + 1) * 512
            clen = min(nxt, Q1) - off
            chunks.append((j, off, clen))
            off += clen
    return chunks


def _emit(tc, xt_h, w_h, wo_h, out_h, causal, dd):
    nc = tc.nc
    nd = (dd + 127) // 128          # number of contraction sub-tiles
    dsubs = [(i * 128, min(128, dd - i * 128)) for i in range(nd)]

    with (
        tc.tile_pool(name="persist", bufs=1) as pp,
        tc.tile_pool(name="pt", bufs=2) as pt_pool,
        tc.tile_pool(name="norm", bufs=2) as norm_pool,
        tc.tile_pool(name="outsb", bufs=3) as out_pool,
        tc.tile_pool(name="dram", bufs=2, space="DRAM") as dram_pool,
    ):
        # ---- persistent SBUF tensors ----
        xt_sb = pp.tile([128, nd, S], BF16, tag="xt", name="xt_sb")
        w_sb = pp.tile([128, nd, E], BF16, tag="w", name="w_sb")
        wo_sb = pp.tile([128, 2, D], BF16, tag="wo", name="wo_sb")
        qkT_sb = pp.tile([128, 4, S], BF16, tag="qkT", name="qkT_sb")
        # V' per (k-tile j, head h): [128, 65], col 64 = ones (softmax denom)
        vp_sb = pp.tile([128, 16, HPC, 65], BF16, tag="vp", name="vp_sb")
        ctx_all = pp.tile([128, 2, S], BF16, tag="ctx", name="ctx_all")

        warm_sb = pp.tile([128, 128], BF16, tag="warm", name="warm_sb")
        nc.vector.memset(warm_sb, 0.0)
        # Pre-load the exp activation table set now (~2.7us); doing it
        # lazily at the first softmax exp opens a >3.4us PE-idle window at
        # the proj->attention boundary and re-throttles the PE clock.
        exp_pre = pp.tile([128, 8], F32, tag="exppre", name="exp_pre")
        nc.scalar.activation(exp_pre, warm_sb[:, 0:8], EXPFN)

        # d-major DMA bundles to match the e-tile d-loop consumption order.
        # xt lands in column halves: the low half (queries/keys 0..1023)
        # unlocks the half-0 attention round ~8us earlier.
        for d, (o, ln) in enumerate(dsubs):
            nc.sync.dma_start(out=w_sb[0:ln, d, :], in_=w_h[o : o + ln, :])
            nc.sync.dma_start(
                out=xt_sb[0:ln, d, 0:HQ], in_=xt_h[o : o + ln, 0:HQ]
            )
        for d, (o, ln) in enumerate(dsubs):
            nc.sync.dma_start(
                out=xt_sb[0:ln, d, HQ:S], in_=xt_h[o : o + ln, HQ:S]
            )
        for et in range(2):
            nc.sync.dma_start(
                out=wo_sb[:, et, :], in_=wo_h[128 * et : 128 * (et + 1), :]
            )

        if causal:
            tri_f = pp.tile([128, 128], F32, tag="trif", name="tri_f")
            make_upper_triangular(nc, tri_f, val=1.0, diag=True)
            tri_bf = pp.tile([128, 128], BF16, tag="trib", name="tri_bf")
            nc.vector.tensor_copy(tri_bf, tri_f)

        nc.vector.memset(vp_sb[:, :, :, 64:65], 1.0)
        # ones row at partition 64: K=1 stationary operand broadcasting the
        # softmax denominator row (also on partition 64) across 64 partitions
        ones_t = pp.tile([65, 64], F32, tag="ones", name="ones_t")
        nc.vector.memset(ones_t[64:65, :], 1.0)
        ones_row = ones_t[64:65, :]

        # ---- era 1: PE warmup on constant data while input DMA streams ----
        with tc.tile_pool(name="wup", bufs=1, space="PSUM") as wup:
            wps = wup.tile([128, 128], F32, tag="w", name="warm_ps")
            for _ in range(96):
                nc.tensor.matmul(wps, lhsT=warm_sb, rhs=warm_sb,
                                 start=True, stop=True)

        # ---- era 2: Q/K proj for pair 0 (e-tiles 0,2) d-outer (DMA-paced),
        #      then V s-tiles 0..7 (enough to unlock half-0 attention) ----
        def emit_v_tile(i, ps, evac_eng):
            for d in range(nd):
                ln = dsubs[d][1]
                nc.tensor.matmul(
                    ps,
                    lhsT=xt_sb[0:ln, d, 128 * i : 128 * (i + 1)],
                    rhs=w_sb[0:ln, d, 2 * EQ : 3 * EQ],
                    start=(d == 0),
                    stop=(d == nd - 1),
                )
            evac_eng(
                vp_sb[:, i, :, 0:64],
                ps.rearrange("p (h e) -> p h e", h=HPC),
            )

        # Only the low-column (queries 0..1023) Q/K of pair 0 plus V tiles
        # 0..7 are needed to start the half-0 attention round; the rest of
        # the projection is woven into the attention pipeline.
        with tc.tile_pool(name="pj", bufs=1, space="PSUM") as filp:
            pss = {}
            for et in (0, 2):
                for ch in range(2):
                    pss[(et, ch)] = filp.tile(
                        [128, 512], F32, tag=f"p{et}c{ch}", name=f"p{et}c{ch}"
                    )
            for d in range(nd):
                ln = dsubs[d][1]
                for et in (0, 2):
                    for ch in range(2):
                        nc.tensor.matmul(
                            pss[(et, ch)],
                            lhsT=w_sb[0:ln, d, 128 * et : 128 * (et + 1)],
                            rhs=xt_sb[0:ln, d, 512 * ch : 512 * (ch + 1)],
                            start=(d == 0),
                            stop=(d == nd - 1),
                        )
            for ch in range(2):
                nc.scalar.copy(
                    qkT_sb[:, 0, 512 * ch : 512 * (ch + 1)], pss[(0, ch)]
                )
                nc.vector.tensor_copy(
                    qkT_sb[:, 2, 512 * ch : 512 * (ch + 1)], pss[(2, ch)]
                )
            for i in range(8):
                vps = filp.tile(
                    [128, 256], F32, tag=f"p0c{i % 2}", name="v_ps"
                )
                emit_v_tile(
                    i, vps, nc.scalar.copy if i % 2 else nc.vector.tensor_copy
                )

        # ---- era 3: softmax pipeline woven with leftover proj + outproj(h0) ----
        with (
            tc.tile_pool(name="stp", bufs=1, space="PSUM") as stp,
            tc.tile_pool(name="ctxp", bufs=1, space="PSUM") as ctxp,
            tc.tile_pool(name="wvp", bufs=1, space="PSUM") as wvp,
        ):
            def weave_qk(ets_chs):
                """512-col chunks of Q/K projection via the weave bank."""
                for et, ch in ets_chs:
                    ps = wvp.tile([128, 512], F32, tag="wv", name="wv_ps")
                    for d in range(nd):
                        ln = dsubs[d][1]
                        nc.tensor.matmul(
                            ps,
                            lhsT=w_sb[0:ln, d, 128 * et : 128 * (et + 1)],
                            rhs=xt_sb[0:ln, d, 512 * ch : 512 * (ch + 1)],
                            start=(d == 0),
                            stop=(d == nd - 1),
                        )
                    nc.vector.tensor_copy(
                        qkT_sb[:, et, 512 * ch : 512 * (ch + 1)], ps
                    )
                    yield

            def weave_v(lo, hi):
                for i in range(lo, hi):
                    ps = wvp.tile([128, 256], F32, tag="wv", name="wv_ps")
                    emit_v_tile(i, ps, nc.vector.tensor_copy)
                    yield

            def weave_outproj(lo, hi):
                """Output projection for query tiles [lo, hi) through the
                1-bank weave slot; needs ctx_all of both pairs normalized."""
                for i in range(lo, hi):
                    osb = out_pool.tile([128, D], BF16, tag="osb", name="o_sb")
                    for c in range(2):
                        ps = wvp.tile([128, 512], F32, tag="wv", name="wv_ps")
                        for et in range(2):
                            nc.tensor.matmul(
                                ps,
                                lhsT=ctx_all[:, et, 128 * i : 128 * (i + 1)],
                                rhs=wo_sb[:, et, 512 * c : 512 * (c + 1)],
                                start=(et == 0),
                                stop=(et == 1),
                            )
                        nc.vector.tensor_copy(osb[:, 512 * c : 512 * (c + 1)], ps)
                        yield
                    nc.sync.dma_start(
                        out=out_h[128 * i : 128 * (i + 1), :], in_=osb
                    )

            def attn_head(p, hh, half, weave_iter, do_norm=True):
                hidx = 2 * p + hh
                ksl = qkT_sb[hh * 64 : (hh + 1) * 64, 2 + p, :]
                qsl = qkT_sb[hh * 64 : (hh + 1) * 64, p, :]
                Q0, Q1 = HQ * half, HQ * (half + 1)
                chunks = _build_chunks(Q0, Q1, causal)
                ctx_ps = ctxp.tile([65, HQ], F32, tag="ctx", name="ctx_ps")
                # Exp groups of up to 2 chunks (big ACT calls amortize the
                # ~300ns per-call overhead) over a 3-deep ring of separate
                # tiles (2+2+1 PSUM banks).  Separate tiles sidestep Tile's
                # coarse cross-engine PSUM WAR tracking; depth 3 hides the
                # scores->exp->scores semaphore latency.  No junk matmuls:
                # an exp-dependent PE instruction ahead of the next scores
                # serializes the pipeline (measured), and ~90% PE duty is
                # enough to hold the HAM clock warm.
                caps = (2, 2, 1)
                sts = [
                    stp.tile([128, caps[r], 512], F32, tag=f"st{r}",
                             name=f"st{r}")
                    for r in range(3)
                ]
                ptss = [
                    pt_pool.tile([128, caps[r], 512], BF16, tag=f"pts{r}",
                                 name=f"pts{r}")
                    for r in range(3)
                ]
                # bucket-sort chunks by length (desc) and pack into the
                # [2,2,1] group-size cycle so paired chunks share a length
                buckets = {}
                for c in chunks:
                    buckets.setdefault(c[2], []).append(c)
                stream = [c for L in sorted(buckets, reverse=True)
                          for c in buckets[L]]
                groups = []
                i = 0
                while i < len(stream):
                    n = min(caps[len(groups) % 3], len(stream) - i)
                    groups.append(stream[i : i + n])
                    i += n

                bank_cnt = [0, 0]
                for (j, qoff, clen) in chunks:
                    bank_cnt[(qoff - Q0) // 512] += 1
                bank_started = [False, False]

                def emit_avs(grp, pts):
                    for m, (j, qoff, clen) in enumerate(grp):
                        b = (qoff - Q0) // 512
                        first = not bank_started[b]
                        bank_started[b] = True
                        bank_cnt[b] -= 1
                        nc.tensor.matmul(
                            ctx_ps[:, qoff - Q0 : qoff - Q0 + clen],
                            lhsT=vp_sb[:, j, hidx, :],
                            rhs=pts[:, m, 0:clen],
                            start=first,
                            stop=(bank_cnt[b] == 0),
                        )

                pend = deque()
                for gi, grp in enumerate(groups):
                    st = sts[gi % 3]
                    pts = ptss[gi % 3]
                    n = len(grp)
                    L = max(c[2] for c in grp)
                    for m, (j, qoff, clen) in enumerate(grp):
                        nc.tensor.matmul(
                            st[:, m, 0:clen],
                            lhsT=ksl[:, 128 * j : 128 * (j + 1)],
                            rhs=qsl[:, qoff : qoff + clen],
                            start=True,
                            stop=True,
                        )
                    nc.scalar.activation(
                        pts[:, 0:n, 0:L], st[:, 0:n, 0:L], EXPFN, scale=0.125
                    )
                    if causal:
                        for m, (j, qoff, clen) in enumerate(grp):
                            if 128 * j >= Q0 and qoff == 128 * j:
                                nc.gpsimd.tensor_mul(
                                    pts[:, m, 0:128], pts[:, m, 0:128], tri_bf
                                )
                    pend.append((grp, pts))
                    if len(pend) >= 2:
                        emit_avs(*pend.popleft())
                    if weave_iter is not None:
                        next(weave_iter, None)
                while pend:
                    emit_avs(*pend.popleft())

                # normalization: evacuate unnormalized ctx^T (+ denom row 64),
                # broadcast the denominator row to 64 partitions with a K=1
                # matmul (no DRAM bounce: 4 DMA hops cost ~8us of latency,
                # fully exposed on the final pass), reciprocal on DVE, mul.
                # Normalization stays entirely off the PE: a PE instruction
                # here (e.g. a K=1 broadcast matmul) would sit in the PE
                # FIFO gated by the DVE evac and head-of-line block the next
                # pass's scores (measured: catastrophic).  The denominator
                # row bounces through DRAM to reshape [128, HQ/128] so the
                # reciprocal uses all DVE lanes, then broadcasts back.
                ctxu = norm_pool.tile([65, HQ], F32, tag="ctxu", name="ctxu")
                nc.vector.tensor_copy(ctxu, ctx_ps)
                if not do_norm:
                    # final pass: normalization happens in era 4 via a
                    # broadcast matmul (nothing left to head-of-line block)
                    return ctxu, ptss[(len(groups) - 1) % 3], p, hh, Q0, Q1
                den_d = dram_pool.tile([HQ], F32, tag="dend", name="den_d")
                nc.sync.dma_start(out=den_d, in_=ctxu[64:65, :])
                den_sp = norm_pool.tile(
                    [128, HQ // 128], F32, tag="densp", name="den_sp"
                )
                nc.sync.dma_start(
                    out=den_sp, in_=den_d.rearrange("(p i) -> p i", p=128)
                )
                rec_sp = norm_pool.tile(
                    [128, HQ // 128], F32, tag="recsp", name="rec_sp"
                )
                nc.vector.reciprocal(rec_sp, den_sp)
                rec_d = dram_pool.tile([HQ], F32, tag="recd", name="rec_d")
                nc.sync.dma_start(out=rec_d, in_=rec_sp)
                recb = norm_pool.tile([64, HQ], F32, tag="recb", name="recb")
                rec_bcast = bass.AP(
                    tensor=rec_d.tensor, offset=rec_d.offset,
                    ap=[[0, 64]] + list(rec_d.ap),
                )
                nc.sync.dma_start(out=recb, in_=rec_bcast)
                nc.vector.tensor_mul(
                    ctx_all[64 * hh : 64 * hh + 64, p, Q0:Q1],
                    ctxu[0:64, :],
                    recb,
                )
                return ptss[(len(groups) - 1) % 3]

            def chain(*gens):
                for g in gens:
                    yield from g

            # weave supply, ordered by when consumers need it:
            #  A: V tiles 2..7 (pass A's own AVs, pipelined just-in-time)
            #  B: pair-1 lo Q/K (needed by passes C,D)
            #  C,D: pair-0 hi Q/K (for E,F) + V tiles 8..15 (for any h1)
            #  E..H: pair-1 hi Q/K (for G,H), then the h0 output projection
            w_ab = weave_qk([(1, 0), (1, 1), (3, 0), (3, 1)])
            w_cd = chain(
                weave_qk([(0, 2), (0, 3), (2, 2), (2, 3)]), weave_v(8, 16)
            )
            w_eh = chain(
                weave_qk([(1, 2), (1, 3), (3, 2), (3, 3)]),
                weave_outproj(0, 8),
            )
            attn_head(0, 0, 0, w_ab)
            attn_head(0, 1, 0, w_ab)
            attn_head(1, 0, 0, w_cd)
            attn_head(1, 1, 0, w_cd)
            attn_head(0, 0, 1, w_eh)
            attn_head(0, 1, 1, w_eh)
            attn_head(1, 0, 1, w_eh)
            last = attn_head(1, 1, 1, w_eh, do_norm=False)
            for it in (w_ab, w_cd, w_eh):
                for _ in it:   # drain leftovers
                    pass

        # ---- era 4: output projection for half 1 ----
        with tc.tile_pool(name="op", bufs=3, space="PSUM") as op:
            l_ctxu, l_pts, l_p, l_hh, l_Q0, l_Q1 = last
            # bridge the final evac/norm window with junk matmuls pinned
            # behind the final exp, keeping the clock warm for era 4
            jt = op.tile([128, D], F32, tag="o", name="jt_ps")
            for _ in range(50):
                nc.tensor.matmul(
                    jt[:, 0:128], lhsT=warm_sb, rhs=l_pts[:, 0, 0:128],
                    start=True, stop=True,
                )
            # final-pass normalization: K=1 broadcast matmul + DVE reciprocal
            # (saves the ~5us DRAM-bounce latency on the critical tail)
            bc = op.tile([64, HQ], F32, tag="o", name="den_bc")
            for c in range(2):
                nc.tensor.matmul(
                    bc[:, 512 * c : 512 * (c + 1)],
                    lhsT=ones_row,
                    rhs=l_ctxu[64:65, 512 * c : 512 * (c + 1)],
                    start=True,
                    stop=True,
                )
                recb = norm_pool.tile([64, 512], F32, tag="recb", name="recb")
                nc.vector.reciprocal(recb, bc[:, 512 * c : 512 * (c + 1)])
                nc.vector.tensor_mul(
                    ctx_all[
                        64 * l_hh : 64 * l_hh + 64,
                        l_p,
                        l_Q0 + 512 * c : l_Q0 + 512 * (c + 1),
                    ],
                    l_ctxu[0:64, 512 * c : 512 * (c + 1)],
                    recb,
                )
            for i in range(8, 16):
                ops = op.tile([128, D], F32, tag="o", name="o_ps")
                for c in range(2):
                    for et in range(2):
                        nc.tensor.matmul(
                            ops[:, 512 * c : 512 * (c + 1)],
                            lhsT=ctx_all[:, et, 128 * i : 128 * (i + 1)],
                            rhs=wo_sb[:, et, 512 * c : 512 * (c + 1)],
                            start=(et == 0),
                            stop=(et == 1),
                        )
                osb = out_pool.tile([128, D], BF16, tag="osb", name="o_sb")
                if i % 2:
                    nc.scalar.copy(osb, ops)
                else:
                    nc.vector.tensor_copy(osb, ops)
                nc.sync.dma_start(out=out_h[128 * i : 128 * (i + 1), :], in_=osb)


def _get_prog(causal: bool, dd: int):
    key = (causal, dd)
    if key not in _prog_cache:
        nc = bacc.Bacc("TRN2", target_bir_lowering=False, debug=False)
        xt_h = nc.dram_tensor("xt", [dd, S], BF16, kind="ExternalInput")
        w_h = nc.dram_tensor("w", [dd, E], BF16, kind="ExternalInput")
        wo_h = nc.dram_tensor("wo", [EQ, D], BF16, kind="ExternalInput")
        out_h = nc.dram_tensor("out", [S, D], BF16, kind="ExternalOutput")
        with tile.TileContext(nc) as tc:
            _emit(tc, xt_h, w_h, wo_h, out_h, causal, dd)
        nc.compile()
        _prog_cache[key] = nc
    return _prog_cache[key]


def _numpy_fallback(x, mask, qkv_w, qkv_b, out_w, out_b):
    qkv = x.reshape(B * S, D) @ qkv_w + qkv_b
    qkv = qkv.reshape(B, S, 3, H, DH)
    q, k, v = qkv[:, :, 0], qkv[:, :, 1], qkv[:, :, 2]
    sc = np.einsum("bqhd,bkhd->bhqk", q, k) / np.sqrt(np.float32(DH))
    sc = np.where(mask, sc, np.float32(-1e9))
    sc = sc - sc.max(-1, keepdims=True)
    a = np.exp(sc)
    a = a / a.sum(-1, keepdims=True)
    ctx = np.einsum("bhqk,bkhd->bqhd", a, v).reshape(B, S, D)
    return (ctx.reshape(B * S, D) @ out_w + out_b).reshape(B, S, D).astype(np.float32)


def kernel(x, mask, qkv_w, qkv_b, out_w, out_b):
    global last_results
    x = np.asarray(x, dtype=np.float32)
    mask = np.asarray(mask).astype(bool)
    qkv_w = np.asarray(qkv_w, dtype=np.float32)
    qkv_b = np.asarray(qkv_b, dtype=np.float32)
    out_w = np.asarray(out_w, dtype=np.float32)
    out_b = np.asarray(out_b, dtype=np.float32)

    m2 = mask.reshape(S, S)
    if m2.all():
        causal = False
    elif np.array_equal(m2, np.tril(np.ones((S, S), dtype=bool))):
        causal = True
    else:
        return _numpy_fallback(x, mask, qkv_w, qkv_b, out_w, out_b)

    has_b = bool(np.any(qkv_b))
    dd = D + 1 if has_b else D
    nc = _get_prog(causal, dd)

    in_maps = []
    for c in range(NCORES):
        b, hg = divmod(c, 4)
        hs = hg * HPC
        cols = slice(hs * DH, (hs + HPC) * DH)
        wc = np.concatenate(
            [qkv_w[:, cols], qkv_w[:, D:][:, cols], qkv_w[:, 2 * D :][:, cols]], axis=1
        )
        xtc = x[b].T
        if has_b:
            bc = np.concatenate(
                [qkv_b[cols], qkv_b[D:][cols], qkv_b[2 * D :][cols]]
            )
            wc = np.concatenate([wc, bc[None, :]], axis=0)
            xtc = np.concatenate([xtc, np.ones((1, S), np.float32)], axis=0)
        in_maps.append(
            {
                "xt": np.ascontiguousarray(xtc).astype(NP_BF16),
                "w": np.ascontiguousarray(wc).astype(NP_BF16),
                "wo": np.ascontiguousarray(out_w[cols, :]).astype(NP_BF16),
            }
        )

    trace = os.environ.get("KERNEL_TRACE", "0") == "1"
    last_results = run_bass_kernel_spmd(
        nc, in_maps, core_ids=list(range(NCORES)), trace=trace
    )
    out = np.zeros((B, S, D), dtype=np.float32)
    for c in range(NCORES):
        out[c // 4] += np.asarray(last_results.results[c]["out"], dtype=np.float32)
    out += out_b[None, None, :]
    return out
